# revision 81
# baseline (speedup 1.0000x reference)
"""HALE attention on 8 Trainium2 NeuronCores — bf16 rewrite.

Sharding: 2 heads/core (tensor-parallel), sequence-parallel tail after a
split AllToAll (core c owns rows [128c:128c+128] and [1024+128c:+128]).

Key layout choices vs the fp32 baseline:
  - all matmul operands bf16 (fp32 PSUM accumulation): single-pass MMs,
    FWL weight loads, half DMA bytes.
  - x^T staged on the host ([128, 8, 2048]); no on-device x transposes.
  - natural-orientation tensors built with PE transposes + PSUM copies
    (XBAR dma_start_transpose costs ~1.2us of issuing-queue time each and
    corrupts data for <128-partition sources, so it is not used).
  - Haar block means + level projections batched into 512-col matmuls with
    the level dim stacked along free cols.
  - augmented-value tiles padded 65->68 cols so PSUM rows stay 8B-aligned.
  - AllToAll split in two (chunks 0-7 / 8-15); first half overlaps the
    second half of the recurrence.

Scheduling notes (365us -> ~308us):
  - recurrence matmuls grouped by weight-tile shape, with the A@V / q@S
    pair interleaved across the two heads per level: consecutive
    same-shape matmuls keep LDWEIGHTS shadowed (54-107ns periods vs
    ~165ns when shapes alternate). Each PSUM region's accumulation group
    must stay contiguous per bank (opening 5 groups on one bank before
    closing any corrupts results on hardware, though the sim accepts it).
  - gpsimd cannot access PSUM and has no TensorScalarPtr; it takes the
    SBUF-only work (P-mask h1, loc-diff, a2a stores). The scalar engine
    drains pso/psSd PSUM->SBUF (bf16) so the per-level combines read
    SBUF at 2x 16-bit DVE rate.
  - startup DMAs reordered critical-path-first across the three DGE
    queues; a tiny warm-up AllToAll absorbs the CC stream's ~11.5us
    first-use setup so the real a2aA starts ~1us after trigger.
  - tail split into per-128-row passes: the gate x-part matmuls (no a2a
    dependency) and pass-0 overlap the second AllToAll; gate/out weights
    stream on the sync queue during the recurrence (16+8 resident
    tiles), with xT's 32KB freed after staging xslT to make room.
"""

import numpy as np
import ml_dtypes
from contextlib import ExitStack

import concourse.bass as bass
import concourse.bacc as bacc
import concourse.tile as tile
import concourse.mybir as mybir
from concourse.bass_utils import run_bass_kernel_spmd

F32 = mybir.dt.float32
BF16 = mybir.dt.bfloat16
AF = mybir.ActivationFunctionType
OP = mybir.AluOpType
BF = ml_dtypes.bfloat16

NCORES = 8
N = 2048
DM = 1024
H = 16
DH = 64
L = 4
CH = 128
NCH = N // CH
WIN = 64
NSL = N // NCORES
EPS = 1e-6
VA = 68  # padded aug-value width (65 used, 68 for 8B PSUM row alignment)

_CACHE = {}
DEBUG = False


def _host_constants():
    ck = np.arange(CH)[:, None]
    cq = np.arange(CH)[None, :]
    maskT = (ck <= cq).astype(np.float32)
    prev = (ck >= cq + WIN + 1).astype(np.float32)
    cur = ((ck <= cq) & (ck >= cq - (WIN - 1))).astype(np.float32)
    lmask = np.concatenate([prev, cur], axis=1)
    Ml = np.zeros((L, CH, CH), dtype=np.float32)
    for lv in range(L):
        b = 2 ** (lv + 1)
        m = np.arange(CH)[:, None]
        n = np.arange(CH)[None, :]
        Ml[lv] = np.where(((m // b) == (n // b)) & (m <= n),
                          1.0 / (n % b + 1.0), 0.0)
    maskT4 = np.tile(maskT, (1, 5))
    Ml_all = Ml.transpose(1, 0, 2).reshape(CH, L * CH)
    return maskT4, lmask, Ml_all


def _blockdiag2(a):
    z = np.zeros((128, 128), dtype=np.float32)
    z[:64, :64] = a
    z[64:, 64:] = a
    return z


def _build_nc():
    nc = bacc.Bacc("TRN2", target_bir_lowering=False, debug=False,
                   num_devices=NCORES)

    xT_d = nc.dram_tensor("xT", [128, 8 * N], BF16, kind="ExternalInput")
    wproj_d = nc.dram_tensor("wproj", [128, 3 * 8 * 128], BF16,
                             kind="ExternalInput")
    wnat_d = nc.dram_tensor("wnat", [128, 8 * 256], BF16,
                            kind="ExternalInput")
    bdWkT_d = nc.dram_tensor("bdWkT", [128, L * 128], BF16,
                             kind="ExternalInput")
    bdWvT_d = nc.dram_tensor("bdWvT", [128, L * 128], BF16,
                             kind="ExternalInput")
    Ml_d = nc.dram_tensor("Ml", [128, L * 128], BF16, kind="ExternalInput")
    maskT4_d = nc.dram_tensor("maskT4", [128, 640], BF16,
                              kind="ExternalInput")
    lmask_d = nc.dram_tensor("lmask", [128, 256], BF16, kind="ExternalInput")
    ident_d = nc.dram_tensor("ident", [128, 128], BF16, kind="ExternalInput")
    w5b_d = nc.dram_tensor("w5b", [128, 5], F32, kind="ExternalInput")
    wgT_d = nc.dram_tensor("wgT", [2 * DM, DM], BF16, kind="ExternalInput")
    woT_d = nc.dram_tensor("woT", [DM, DM], BF16, kind="ExternalInput")
    wgo8_d = nc.dram_tensor("wgo8", [128, 8], BF16, kind="ExternalInput")
    bg_d = nc.dram_tensor("bg", [1, DM], BF16, kind="ExternalInput")
    bo_d = nc.dram_tensor("bo", [1, DM], BF16, kind="ExternalInput")
    bgo_d = nc.dram_tensor("bgo", [128, 1], F32, kind="ExternalInput")
    out_d = nc.dram_tensor("out", [2 * 128, DM], F32, kind="ExternalOutput")

    dbg_d = {}
    if DEBUG:
        for nm, w in (("qT", N), ("kpT", N), ("knat", N), ("kpnat", N),
                      ("vnat", N), ("vlnat", N), ("kplT", L * N),
                      ("kplN", L * N), ("vlvA", L * NCH * 2 * VA),
                      ("vaug", NCH * 2 * VA), ("glob", N), ("loc", N),
                      ("S", 5 * VA), ("atm", 640)):
            dbg_d[nm] = nc.dram_tensor(f"dbg_{nm}", [128, w], BF16,
                                       kind="ExternalOutput")
        for nm, w in (("pso", 5 * VA), ("dmax", 5), ("rw", 5)):
            dbg_d[nm] = nc.dram_tensor(f"dbg_{nm}", [128, w], F32,
                                       kind="ExternalOutput")

    # [dest core, tensor(diff,glob), 128, 128] for chunk half A; half B is
    # split into two collectives (diff first) so the tail's second pass can
    # start its gate work before the glob half arrives.
    a2aA_in = nc.dram_tensor("a2aA_in", [NCORES, 2, 128, 128], BF16)
    a2aA_out = nc.dram_tensor("a2aA_out", [NCORES, 2, 128, 128], BF16)
    a2aBd_in = nc.dram_tensor("a2aBd_in", [NCORES, 128, 128], BF16)
    a2aBd_out = nc.dram_tensor("a2aBd_out", [NCORES, 128, 128], BF16)
    a2aBg_in = nc.dram_tensor("a2aBg_in", [NCORES, 128, 128], BF16)
    a2aBg_out = nc.dram_tensor("a2aBg_out", [NCORES, 128, 128], BF16)
    # tiny warm-up collective to absorb the CC stream's first-use setup
    a2aW_in = nc.dram_tensor("a2aW_in", [NCORES, 16], BF16)
    a2aW_out = nc.dram_tensor("a2aW_out", [NCORES, 16], BF16)

    with tile.TileContext(nc) as tc, ExitStack() as root:
        cpool = root.enter_context(tc.tile_pool(name="consts", bufs=1))
        persist = root.enter_context(tc.tile_pool(name="persist", bufs=1))

        maskT4 = cpool.tile([128, 640], BF16)
        lmask = cpool.tile([128, 256], BF16)
        Ml_sb = cpool.tile([128, L * 128], BF16)
        bdWkT = cpool.tile([128, L * 128], BF16)
        bdWvT = cpool.tile([128, L * 128], BF16)
        ident = cpool.tile([128, 128], BF16)
        w5b = cpool.tile([128, 5], F32)
        ones_row = cpool.tile([1, 128], BF16)
        bg_sb = cpool.tile([1, DM], BF16)
        bo_sb = cpool.tile([1, DM], BF16)
        bgo_sb = cpool.tile([128, 1], F32)
        wgo_sb = cpool.tile([128, 8], BF16)

        glob = persist.tile([128, N], BF16)
        loc = persist.tile([128, N], BF16)
        S_bf = persist.tile([128, 5, VA], BF16)
        xslT = persist.tile([128, 8, 2, 128], BF16)
        diff_gs = persist.tile([128, 2, DM], BF16)
        glob_gs = persist.tile([128, 2, DM], BF16)
        recur = persist
        # xT lives in its own stack-scoped pool: freed once xslT is staged
        xtp = ExitStack()
        xT = xtp.enter_context(
            tc.tile_pool(name="xtp", bufs=1)).tile([128, 8, N], BF16)
        qT = recur.tile([128, N], BF16)
        klT = recur.tile([128, N], BF16)
        qpT = recur.tile([128, N], BF16)
        kpT = recur.tile([128, N], BF16)
        kpnat = recur.tile([128, N], BF16)
        kplT = recur.tile([128, L, N], BF16)
        kplN = recur.tile([128, L, N], BF16)
        vaug = recur.tile([128, NCH, 2, VA], BF16)
        vlaug = recur.tile([128, NCH, 2, VA], BF16)
        vlvA = recur.tile([128, L, NCH, 2, VA], BF16)
        if DEBUG:
            dpso = persist.tile([128, 5, VA], F32)
            datm = persist.tile([128, 640], BF16)
            ddmax = persist.tile([128, 5], F32)
            drw = persist.tile([128, 5], F32)

        nc.vector.memset(vaug[:, :, :, 64:65], 1.0)
        nc.vector.memset(vlaug[:, :, :, 64:65], 1.0)
        nc.vector.memset(vlvA[:, :, :, :, 64:65], 1.0)

        def cp(dst, src, on_scalar=False):
            if on_scalar:
                nc.scalar.activation(dst, src, AF.Copy)
            else:
                nc.vector.tensor_copy(dst, src)

        def cp3(dst, src, eng):
            # gpsimd cannot touch PSUM; rotate psum->sbuf copies over V/S only
            if eng % 2 == 0:
                nc.vector.tensor_copy(dst, src)
            else:
                nc.scalar.activation(dst, src, AF.Copy)

        with ExitStack() as phA:
            nat_p = phA.enter_context(tc.tile_pool(name="nat", bufs=1))
            knat = nat_p.tile([128, N], BF16)
            vnat = nat_p.tile([128, N], BF16)
            vlnat = nat_p.tile([128, N], BF16)
            tmp_p = phA.enter_context(tc.tile_pool(name="phitmp", bufs=2))
            phPrj = phA.enter_context(ExitStack())
            prj = phPrj.enter_context(tc.tile_pool(name="prj", bufs=1))
            phP = phA.enter_context(ExitStack())
            ps_pr = phP.enter_context(
                tc.tile_pool(name="ps_pr", bufs=2, space="PSUM"))

            wproj = prj.tile([128, 3 * 8 * 128], BF16)
            wnat = prj.tile([128, 8, 256], BF16)
            kT = prj.tile([128, N], BF16)

            # ---- DMA issue order: critical-path loads first, spread over
            # the DGE queues; per-projection wproj slices so the first
            # matmul only waits on 0.4MB, not the whole tensor.
            for ip in range(3):
                nc.sync.dma_start(wproj[:, 1024 * ip:1024 * (ip + 1)],
                                  wproj_d[:, 1024 * ip:1024 * (ip + 1)])
            nc.sync.dma_start(wnat[:], wnat_d[:])
            xq = (nc.scalar, nc.gpsimd, nc.sync)
            for k in range(8):
                xq[k % 3].dma_start(xT[:, k, :], xT_d[:, N * k:N * (k + 1)])
            nc.sync.dma_start(ident[:], ident_d[:])
            nc.sync.dma_start(Ml_sb[:], Ml_d[:])
            nc.scalar.dma_start(maskT4[:], maskT4_d[:])
            nc.scalar.dma_start(lmask[:], lmask_d[:])
            nc.gpsimd.dma_start(bdWkT[:], bdWkT_d[:])
            nc.gpsimd.dma_start(bdWvT[:], bdWvT_d[:])
            nc.gpsimd.dma_start(w5b[:], w5b_d[:])
            nc.scalar.dma_start(bg_sb[:], bg_d[:])
            nc.scalar.dma_start(bo_sb[:], bo_d[:])
            nc.gpsimd.dma_start(bgo_sb[:], bgo_d[:])
            nc.gpsimd.dma_start(wgo_sb[:], wgo8_d[:])
            nc.vector.memset(ones_row[:], 1.0)
            nc.gpsimd.collective_compute(
                "AllToAll", OP.bypass,
                ins=[a2aW_in.ap().opt()], outs=[a2aW_out.ap().opt()],
                replica_groups=[list(range(NCORES))])

            def phi_big(dst, src):
                tmp = tmp_p.tile([128, N], BF16, tag="phitmp")
                nc.vector.tensor_scalar_min(tmp[:], src[:], 0.0)
                nc.scalar.activation(dst[:], tmp[:], AF.Exp)
                nc.vector.scalar_tensor_tensor(
                    dst[:], src[:], 0.0, dst[:], op0=OP.max, op1=OP.add)

            # ----- projections (k-outer, 4x512-col accumulators) -----
            for ip, dstT in enumerate((qT, kT, klT)):
                accs = [ps_pr.tile([128, 512], F32, tag=f"pa{nb}",
                                   name=f"acc{nb}")
                        for nb in range(4)]
                for k in range(8):
                    for nb in range(4):
                        nc.tensor.matmul(
                            accs[nb][:], wproj[:, (ip * 8 + k) * 128:
                                                (ip * 8 + k + 1) * 128],
                            xT[:, k, 512 * nb:512 * (nb + 1)],
                            start=(k == 0), stop=(k == 7))
                for nb in range(4):
                    cp(dstT[:, 512 * nb:512 * (nb + 1)], accs[nb][:],
                       on_scalar=(nb % 2 == 1))
                if ip == 0:
                    phi_big(qpT, qT)
                elif ip == 1:
                    phi_big(kpT, kT)


            # ----- Haar: block means + level projections (batched) -----
            phP.close()
            with ExitStack() as trA:
                # v / v_loc computed directly in natural [seq, d] orientation
                # (x^T chunk as stationary weights, W columns moving): no
                # transposes, no extra PSUM round-trips.
                ps_nat = trA.enter_context(
                    tc.tile_pool(name="ps_nat", bufs=3, space="PSUM"))
                ps_tr1 = trA.enter_context(
                    tc.tile_pool(name="ps_tr1", bufs=3, space="PSUM"))
                for i in range(NCH):
                    sl = slice(CH * i, CH * (i + 1))
                    pn = ps_nat.tile([128, 256], F32, tag="nat")
                    for k in range(8):
                        nc.tensor.matmul(pn[:], xT[:, k, sl], wnat[:, k, :],
                                         start=(k == 0), stop=(k == 7))
                    cp3(vnat[:, sl], pn[:, 0:128], i)
                    cp3(vlnat[:, sl], pn[:, 128:256], i + 1)
                    ptn = ps_tr1.tile([128, 128], BF16, tag="ptk")
                    nc.tensor.transpose(ptn[:], kT[:, sl], ident[:])
                    cp3(knat[:, sl], ptn[:], i)
                    nc.gpsimd.tensor_copy(
                        vaug[:, i, :, 0:64],
                        vnat[:, sl].rearrange("p (h d) -> p h d", h=2))
                    nc.gpsimd.tensor_copy(
                        vlaug[:, i, :, 0:64],
                        vlnat[:, sl].rearrange("p (h d) -> p h d", h=2))
            bm_p = phA.enter_context(tc.tile_pool(name="bm", bufs=2))
            ps_bm = phA.enter_context(
                tc.tile_pool(name="ps_bm", bufs=1, space="PSUM"))
            ps_trH = phA.enter_context(
                tc.tile_pool(name="ps_trH", bufs=2, space="PSUM"))
            phi_big(kpnat, knat)
            for g in range(4):
                bmk = bm_p.tile([128, 4, L, 128], BF16, tag="bmk", bufs=2)
                bmv = bm_p.tile([128, 4, L, 128], BF16, tag="bmv", bufs=2)
                for ig in range(4):
                    i = 4 * g + ig
                    sl = slice(CH * i, CH * (i + 1))
                    psk = ps_bm.tile([128, 512], F32, tag="bmk", bufs=2)
                    nc.tensor.matmul(psk[:], knat[:, sl], Ml_sb[:],
                                     start=True, stop=True)
                    nc.vector.tensor_copy(bmk[:, ig], psk[:].rearrange(
                        "p (l s) -> p l s", l=L))
                    psv = ps_bm.tile([128, 512], F32, tag="bmv")
                    nc.tensor.matmul(psv[:], vnat[:, sl], Ml_sb[:],
                                     start=True, stop=True)
                    nc.scalar.activation(bmv[:, ig], psv[:].rearrange(
                        "p (l s) -> p l s", l=L), AF.Copy)
                nsl = slice(512 * g, 512 * (g + 1))
                for lv in range(L):
                    pst = ps_bm.tile([128, 512], F32, tag="kt", bufs=1)
                    nc.tensor.matmul(pst[:],
                                     bdWkT[:, 128 * lv:128 * (lv + 1)],
                                     bmk[:, :, lv, :], start=True, stop=True)
                    tmp = tmp_p.tile([128, 512], BF16, tag="phs")
                    nc.vector.tensor_scalar_min(tmp[:], pst[:], 0.0)
                    nc.scalar.activation(kplT[:, lv, nsl], tmp[:], AF.Exp)
                    nc.vector.scalar_tensor_tensor(
                        kplT[:, lv, nsl], pst[:], 0.0, kplT[:, lv, nsl],
                        op0=OP.max, op1=OP.add)
                for lv in range(L):
                    for ig in range(4):
                        i = 4 * g + ig
                        sl = slice(CH * i, CH * (i + 1))
                        ptk = ps_trH.tile([128, 128], BF16, tag="ptk")
                        nc.tensor.transpose(ptk[:], kplT[:, lv, sl], ident[:])
                        cp3(kplN[:, lv, sl], ptk[:], 2 * ig)
                        # vlv directly in natural [seq, d] orientation:
                        # contraction over the block-mean feature partitions
                        pv = ps_trH.tile([128, 128], F32, tag="ptv")
                        nc.tensor.matmul(pv[:], bmv[:, ig, lv, :],
                                         bdWvT[:, 128 * lv:128 * (lv + 1)],
                                         start=True, stop=True)
                        cp3(vlvA[:, lv, i, :, 0:64],
                            pv[:].rearrange("p (h d) -> p h d", h=2),
                            2 * ig + 1)

        # stage this core's x^T rows, then free xT's SBUF for the tail
        pid = nc.sync.partition_id()
        r0 = pid * 128
        xTv = xT[:].rearrange("p k (t n) -> p k t n", t=2)
        nc.sync.dma_start(xslT[:], xTv[:, :, :, bass.ds(r0, 128)])
        xtp.close()

        # ----- chunk-major recurrence + local attention -----
        with ExitStack() as phB:
            atm_p = phB.enter_context(tc.tile_pool(name="atm", bufs=3))
            tin_p = phB.enter_context(tc.tile_pool(name="tiny", bufs=4))
            ps_Aa = phB.enter_context(
                tc.tile_pool(name="ps_Aa", bufs=2, space="PSUM"))
            ps_Ab = phB.enter_context(
                tc.tile_pool(name="ps_Ab", bufs=1, space="PSUM"))
            ps_O = phB.enter_context(
                tc.tile_pool(name="ps_O", bufs=2, space="PSUM"))
            ps_Sd = phB.enter_context(
                tc.tile_pool(name="ps_Sd", bufs=1, space="PSUM"))

            for i in range(NCH):
                sl = slice(CH * i, CH * (i + 1))
                psSd = ps_Sd.tile([128, 7, VA], F32, tag="psSd")
                atms, Ps, psLs = [], [], []
                for h in range(2):
                    hp = slice(64 * h, 64 * h + 64)
                    psa_a = ps_Aa.tile([128, 512], F32, tag="psa")
                    psab = ps_Ab.tile([128, 384], F32, tag="psb", bufs=2)
                    for lv in range(5):
                        lhsT = (kpT[hp, sl] if lv == 0
                                else kplT[hp, lv - 1, sl])
                        out = (psa_a[:, 128 * (lv - 1):128 * lv] if lv > 0
                               else psab[:, 0:128])
                        nc.tensor.matmul(out, lhsT, qpT[hp, sl],
                                         start=True, stop=True)
                    if i > 0:
                        nc.tensor.matmul(psab[:, 128:256],
                                         klT[hp, CH * (i - 1):CH * i],
                                         qT[hp, sl], start=True, stop=True)
                    nc.tensor.matmul(psab[:, 256:384], klT[hp, sl],
                                     qT[hp, sl], start=True, stop=True)
                    ve = nc.vector if h == 0 else nc.gpsimd
                    atm = atm_p.tile([128, 640], BF16, tag="atm")
                    nc.vector.tensor_mul(atm[:, 0:512], psa_a[:],
                                         maskT4[:, 0:512])
                    nc.vector.tensor_mul(atm[:, 512:640], psab[:, 0:128],
                                         maskT4[:, 512:640])
                    P = atm_p.tile([128, 256], BF16, tag="P")
                    if i > 0:
                        nc.scalar.activation(P[:], psab[:, 128:384], AF.Exp,
                                             scale=0.125)
                        nc.gpsimd.tensor_mul(P[:], P[:], lmask[:])
                    else:
                        nc.scalar.activation(P[:, 128:256], psab[:, 256:384],
                                             AF.Exp, scale=0.125)
                        nc.gpsimd.tensor_mul(P[:, 128:256], P[:, 128:256],
                                             lmask[:, 128:256])
                    atms.append(atm)
                    Ps.append(P)
                # ---- A@V + q@S: cross-head interleave per level. Each PSUM
                # region's accumulation group stays contiguous on its bank
                # (hw requirement), but pairing the two heads halves the
                # weight-shape alternation so LDWEIGHTS shadows better.
                psos = [ps_O.tile([128, 5, VA], F32, tag="pso",
                                  name=f"pso{h}", bufs=3) for h in range(2)]
                for lv in range(5):
                    for h in range(2):
                        atm = atms[h]
                        alv = atm[:, 512:640] if lv == 0 else \
                            atm[:, 128 * (lv - 1):128 * lv]
                        va_l = (vaug[:, i, h, 0:65] if lv == 0
                                else vlvA[:, lv - 1, i, h, 0:65])
                        nc.tensor.matmul(psos[h][:, lv, 0:65], alv, va_l,
                                         start=True, stop=(i == 0))
                    if i > 0:
                        for h in range(2):
                            hp = slice(64 * h, 64 * h + 64)
                            nc.tensor.matmul(psos[h][:, lv, 0:65],
                                             qpT[hp, sl], S_bf[hp, lv, 0:65],
                                             start=False, stop=True)
                # group 3: state updates, 128x64 weights (10 instr)
                for h in range(2):
                    hp = slice(64 * h, 64 * h + 64)
                    c0 = CH * i + 64 * h
                    for lv in range(5):
                        kn_l = (kpnat[:, c0:c0 + 64] if lv == 0
                                else kplN[:, lv - 1, c0:c0 + 64])
                        va_l = (vaug[:, i, h, 0:65] if lv == 0
                                else vlvA[:, lv - 1, i, h, 0:65])
                        nc.tensor.matmul(psSd[hp, lv, 0:65], kn_l, va_l,
                                         start=True, stop=True)
                # group 4: local attention out, 128x128 weights (2-4 instr)
                for h in range(2):
                    P = Ps[h]
                    if i > 0:
                        nc.tensor.matmul(psSd[:, 5 + h, 0:65], P[:, 0:128],
                                         vlaug[:, i - 1, h, 0:65],
                                         start=True, stop=False)
                    nc.tensor.matmul(psSd[:, 5 + h, 0:65], P[:, 128:256],
                                     vlaug[:, i, h, 0:65],
                                     start=(i == 0), stop=True)
                # ---- per-head normalization / combine (vector+gpsimd) -----
                for h in range(2):
                    ve = nc.vector if h == 0 else nc.gpsimd
                    c0 = CH * i + 64 * h
                    pso = psos[h]
                    # drain PSUM->SBUF on the scalar engine, combine on V/G
                    po_t = tin_p.tile([128, 5, VA], BF16, tag=f"pot{h}")
                    nc.scalar.activation(po_t[:, :, 0:65], pso[:, :, 0:65],
                                         AF.Copy)
                    lo_t = tin_p.tile([128, VA], BF16, tag=f"lot{h}")
                    nc.scalar.activation(lo_t[:, 0:65], psSd[:, 5 + h, 0:65],
                                         AF.Copy)
                    dmax = tin_p.tile([128, 5], F32, tag=f"dmax{h}")
                    if DEBUG and i == 1 and h == 0:
                        nc.vector.tensor_copy(dpso[:], pso[:])
                        nc.vector.tensor_copy(datm[:], atms[h][:])
                    nc.vector.tensor_scalar_max(dmax[:], po_t[:, :, 64], EPS)
                    rec = tin_p.tile([128, 5], F32, tag=f"rec{h}")
                    nc.vector.reciprocal(rec[:], dmax[:])
                    rw = tin_p.tile([128, 5], F32, tag=f"rw{h}")
                    ve.tensor_mul(rw[:], rec[:], w5b[:])
                    if DEBUG and i == 1 and h == 0:
                        nc.vector.tensor_copy(ddmax[:], dmax[:])
                        nc.vector.tensor_copy(drw[:], rw[:])
                    gsl = glob[:, c0:c0 + 64]
                    nc.vector.tensor_scalar_mul(gsl, po_t[:, 0, 0:64],
                                                rw[:, 0:1])
                    for lv in range(1, 5):
                        nc.vector.scalar_tensor_tensor(
                            gsl, po_t[:, lv, 0:64], rw[:, lv:lv + 1], gsl,
                            op0=OP.mult, op1=OP.add)
                    dm = tin_p.tile([128, 1], F32, tag=f"dm{h}")
                    nc.vector.tensor_scalar_max(dm[:], lo_t[:, 64:65], 1e-30)
                    rl = tin_p.tile([128, 1], F32, tag=f"rl{h}")
                    nc.vector.reciprocal(rl[:], dm[:])
                    nc.vector.tensor_scalar_mul(loc[:, c0:c0 + 64],
                                                lo_t[:, 0:64], rl[:])
                if i == 0:
                    nc.vector.tensor_copy(S_bf[:, :, 0:65],
                                          psSd[:, 0:5, 0:65])
                else:
                    nc.vector.tensor_add(S_bf[:, :, 0:65], S_bf[:, :, 0:65],
                                         psSd[:, 0:5, 0:65])
                # diff in place + stage this chunk for its dest core
                nc.gpsimd.tensor_sub(loc[:, sl], loc[:, sl], glob[:, sl])
                if i < 8:
                    nc.gpsimd.dma_start(a2aA_in.ap()[i % 8, 0], loc[:, sl])
                    nc.gpsimd.dma_start(a2aA_in.ap()[i % 8, 1], glob[:, sl])
                else:
                    nc.gpsimd.dma_start(a2aBd_in.ap()[i % 8], loc[:, sl])
                    nc.gpsimd.dma_start(a2aBg_in.ap()[i % 8], glob[:, sl])
                if i == 7:
                    nc.gpsimd.collective_compute(
                        "AllToAll", OP.bypass,
                        ins=[a2aA_in.ap().opt()], outs=[a2aA_out.ap().opt()],
                        replica_groups=[list(range(NCORES))])
            nc.gpsimd.collective_compute(
                "AllToAll", OP.bypass,
                ins=[a2aBd_in.ap().opt()], outs=[a2aBd_out.ap().opt()],
                replica_groups=[list(range(NCORES))])
            nc.gpsimd.collective_compute(
                "AllToAll", OP.bypass,
                ins=[a2aBg_in.ap().opt()], outs=[a2aBg_out.ap().opt()],
                replica_groups=[list(range(NCORES))])
            nc.gpsimd.dma_start(
                diff_gs[:, 1, :].rearrange("p (s m) -> p s m", s=8),
                a2aBd_out.ap().rearrange("s p m -> p s m"))
            nc.gpsimd.dma_start(
                glob_gs[:, 1, :].rearrange("p (s m) -> p s m", s=8),
                a2aBg_out.ap().rearrange("s p m -> p s m"))

        # ---------- sequence-parallel tail (split per 128-row half) -------
        with ExitStack() as phC:
            tl = phC.enter_context(tc.tile_pool(name="tail", bufs=1))
            wst = phC.enter_context(tc.tile_pool(name="wstream", bufs=8))
            ps_tr2 = phC.enter_context(
                tc.tile_pool(name="ps_tr2", bufs=2, space="PSUM"))
            ps_g = phC.enter_context(
                tc.tile_pool(name="ps_g", bufs=1, space="PSUM"))

            # weight streams: wg ring (8 deep, 16 slices) + wo (8 alive).
            # The DMAs sit on the sync queue, which runs ahead during the
            # recurrence, so the first ring fill overlaps phase B entirely.
            wg_ts = {}
            wo_ts = {}

            # x-part of the gate GEMM for both halves: no a2a dependency,
            # fills the PE while the second AllToAll is in flight.
            psGs = []
            for t2 in range(2):
                psG = [ps_g.tile([128, 512], F32, tag=f"psG{t2}{j}",
                                 name=f"psG{t2}{j}") for j in range(2)]
                psGs.append(psG)
                for kc in range(8):
                    if t2 == 0:
                        wg_t = wst.tile([128, DM], BF16, tag="wg", bufs=16,
                                        name=f"wg{kc}")
                        nc.sync.dma_start(
                            wg_t[:], wgT_d[128 * kc:128 * (kc + 1), :])
                        wg_ts[kc] = wg_t
                    lhs = xslT[:, kc, t2, :]
                    for g2 in range(2):
                        nc.tensor.matmul(
                            psG[g2][:], lhs,
                            wg_ts[kc][:, 512 * g2:512 * (g2 + 1)],
                            start=(kc == 0), stop=False)

            # gather this core's first-half rows as soon as the first
            # collective lands (sync queue, behind the first wg batch)
            nc.sync.dma_start(
                diff_gs[:, 0, :].rearrange("p (s m) -> p s m", s=8),
                a2aA_out.ap()[:, 0].rearrange("s p m -> p s m"))
            nc.sync.dma_start(
                glob_gs[:, 0, :].rearrange("p (s m) -> p s m", s=8),
                a2aA_out.ap()[:, 1].rearrange("s p m -> p s m"))

            for t2 in range(2):
                psG = psGs[t2]

                diffT = tl.tile([128, 8, 128], BF16, tag="dT", name=f"diffT{t2}")
                for k in range(8):
                    pt = ps_tr2.tile([128, 128], BF16, tag="ptr2")
                    nc.tensor.transpose(
                        pt[:], diff_gs[:, t2, 128 * k:128 * (k + 1)],
                        ident[:])
                    cp(diffT[:, k, :], pt[:], on_scalar=(k % 2 == 1))
                for kc in range(8, 16):
                    if t2 == 0:
                        wg_t = wst.tile([128, DM], BF16, tag="wg", bufs=16,
                                        name=f"wg{kc}")
                        nc.sync.dma_start(
                            wg_t[:], wgT_d[128 * kc:128 * (kc + 1), :])
                        wg_ts[kc] = wg_t
                    lhs = diffT[:, kc - 8, :]
                    for g2 in range(2):
                        nc.tensor.matmul(
                            psG[g2][:], lhs,
                            wg_ts[kc][:, 512 * g2:512 * (g2 + 1)],
                            start=False, stop=False)
                gh = tl.tile([128, DM], BF16, tag="gh", name=f"gh{t2}")
                for g2 in range(2):
                    nc.tensor.matmul(
                        psG[g2][:], ones_row[:],
                        bg_sb[:, 512 * g2:512 * (g2 + 1)],
                        start=False, stop=True)
                    nc.scalar.activation(
                        gh[:, 512 * g2:512 * (g2 + 1)], psG[g2][:], AF.Silu)
                ghT = tl.tile([128, 8, 128], BF16, tag="ghT", name=f"ghT{t2}")
                for k in range(8):
                    pt = ps_tr2.tile([128, 128], BF16, tag="ptr2")
                    nc.tensor.transpose(
                        pt[:], gh[:, 128 * k:128 * (k + 1)], ident[:])
                    cp(ghT[:, k, :], pt[:], on_scalar=(k % 2 == 1))
                psAl = ps_tr2.tile([128, 1], F32, tag="psAl")
                for gc in range(8):
                    nc.tensor.matmul(psAl[:, 0:1], ghT[:, gc, :],
                                     wgo_sb[:, gc:gc + 1],
                                     start=(gc == 0), stop=(gc == 7))
                alpha = tl.tile([128, 1], F32, tag="al", name=f"alpha{t2}")
                nc.scalar.activation(alpha[:], psAl[:], AF.Sigmoid,
                                     bias=bgo_sb[:])
                mx = diff_gs[:, t2, :]
                nc.vector.scalar_tensor_tensor(
                    mx, diff_gs[:, t2, :], alpha[:, 0:1], glob_gs[:, t2, :],
                    op0=OP.mult, op1=OP.add)
                mxT = tl.tile([128, 8, 128], BF16, tag="mxT", name=f"mxT{t2}")
                for k in range(8):
                    pt = ps_tr2.tile([128, 128], BF16, tag="ptr2")
                    nc.tensor.transpose(
                        pt[:], diff_gs[:, t2, 128 * k:128 * (k + 1)],
                        ident[:])
                    cp(mxT[:, k, :], pt[:], on_scalar=(k % 2 == 1))
                out_sb = tl.tile([128, DM], F32, tag="out", name=f"out{t2}")
                psF = [ps_g.tile([128, 512], F32, tag=f"psG{t2}{j}",
                                 name=f"psF{t2}{j}") for j in range(2)]
                for kc in range(8):
                    if t2 == 0:
                        wo_t = wst.tile([128, DM], BF16, tag="wo", bufs=8,
                                        name=f"wo{kc}")
                        nc.sync.dma_start(
                            wo_t[:], woT_d[128 * kc:128 * (kc + 1), :])
                        wo_ts[kc] = wo_t
                    for o2 in range(2):
                        nc.tensor.matmul(
                            psF[o2][:], mxT[:, kc, :],
                            wo_ts[kc][:, 512 * o2:512 * (o2 + 1)],
                            start=(kc == 0), stop=False)
                for o2 in range(2):
                    nc.tensor.matmul(
                        psF[o2][:], ones_row[:],
                        bo_sb[:, 512 * o2:512 * (o2 + 1)],
                        start=False, stop=True)
                    cp(out_sb[:, 512 * o2:512 * (o2 + 1)], psF[o2][:],
                       on_scalar=(o2 == 1))
                nc.sync.dma_start(out_d.ap()[128 * t2:128 * (t2 + 1), :],
                                  out_sb[:])

        if DEBUG:
            for nm, t in (("qT", qT), ("kpT", kpT), ("knat", knat),
                          ("kpnat", kpnat), ("vnat", vnat), ("vlnat", vlnat),
                          ("glob", glob), ("loc", loc)):
                nc.gpsimd.dma_start(dbg_d[nm].ap(), t[:])
            nc.gpsimd.dma_start(
                dbg_d["kplT"].ap().rearrange("p (l n) -> p l n", l=L), kplT[:])
            nc.gpsimd.dma_start(
                dbg_d["kplN"].ap().rearrange("p (l n) -> p l n", l=L), kplN[:])
            nc.gpsimd.dma_start(
                dbg_d["vlvA"].ap().rearrange(
                    "p (l c h v) -> p l c h v", l=L, c=NCH, h=2), vlvA[:])
            nc.gpsimd.dma_start(
                dbg_d["vaug"].ap().rearrange(
                    "p (c h v) -> p c h v", c=NCH, h=2), vaug[:])
            nc.gpsimd.dma_start(
                dbg_d["S"].ap().rearrange("p (l v) -> p l v", l=5), S_bf[:])
            nc.gpsimd.dma_start(
                dbg_d["pso"].ap().rearrange("p (l v) -> p l v", l=5), dpso[:])
            nc.gpsimd.dma_start(dbg_d["atm"].ap(), datm[:])
            nc.gpsimd.dma_start(dbg_d["dmax"].ap(), ddmax[:])
            nc.gpsimd.dma_start(dbg_d["rw"].ap(), drw[:])

    nc.compile()
    return nc


def _prep_in_maps(x, Wq, Wk, Wv, Wkl, Wvl, haar_Wk, haar_Wv, haar_scale,
                  Wg, bg, Wgo, bgo, Wo, bo):
    maskT4, lmask, Ml_all = _host_constants()
    x2 = np.asarray(x, dtype=np.float32).reshape(N, DM)
    xT = np.ascontiguousarray(
        x2.reshape(N, 8, 128).transpose(2, 1, 0).reshape(128, 8 * N)
    ).astype(BF)
    bdWkT = np.concatenate(
        [_blockdiag2(np.asarray(haar_Wk[lv], dtype=np.float32).T)
         for lv in range(L)], axis=1)
    bdWvT = np.concatenate(
        [_blockdiag2(np.asarray(haar_Wv[lv], dtype=np.float32).T)
         for lv in range(L)], axis=1)
    hs = np.asarray(haar_scale, dtype=np.float64)
    sw = np.exp(hs - hs.max())
    sw = (sw / sw.sum()).astype(np.float32)
    w5b = np.tile(np.concatenate([[1.0], sw]).astype(np.float32)[None, :],
                  (128, 1))
    shared = {
        "xT": xT,
        "bdWkT": bdWkT.astype(BF), "bdWvT": bdWvT.astype(BF),
        "Ml": Ml_all.astype(BF), "maskT4": maskT4.astype(BF),
        "lmask": lmask.astype(BF),
        "ident": np.eye(128, dtype=np.float32).astype(BF),
        "w5b": w5b,
        "wgT": np.ascontiguousarray(
            np.asarray(Wg, dtype=np.float32).T).astype(BF),
        "woT": np.ascontiguousarray(
            np.asarray(Wo, dtype=np.float32).T).astype(BF),
        "wgo8": np.ascontiguousarray(
            np.asarray(Wgo, dtype=np.float32).reshape(8, 128).T).astype(BF),
        "bg": np.asarray(bg, dtype=np.float32).reshape(1, DM).astype(BF),
        "bo": np.asarray(bo, dtype=np.float32).reshape(1, DM).astype(BF),
        "bgo": np.full((128, 1), np.asarray(bgo, dtype=np.float32).reshape(()),
                       dtype=np.float32),
    }
    in_maps = []
    for c in range(NCORES):
        sc = slice(128 * c, 128 * (c + 1))
        m = dict(shared)
        wp = np.empty((128, 3, 8, 128), dtype=np.float32)
        for ip, W in enumerate((Wq, Wk, Wkl)):
            Wc = np.asarray(W, dtype=np.float32)[sc, :]  # [128 m, 1024 dm]
            wp[:, ip] = Wc.reshape(128, 8, 128).transpose(2, 1, 0)
        m["wproj"] = np.ascontiguousarray(
            wp.reshape(128, 3 * 8 * 128)).astype(BF)
        wn = np.empty((128, 8, 256), dtype=np.float32)
        for j, W in enumerate((Wv, Wvl)):
            Wc = np.asarray(W, dtype=np.float32)[sc, :]  # [128 e, 1024 dm]
            wn[:, :, 128 * j:128 * (j + 1)] = \
                Wc.reshape(128, 8, 128).transpose(2, 1, 0)
        m["wnat"] = np.ascontiguousarray(
            wn.reshape(128, 8 * 256)).astype(BF)
        in_maps.append(m)
    return in_maps


def kernel_run(inputs, trace=False):
    if "nc" not in _CACHE:
        _CACHE["nc"] = _build_nc()
    nc = _CACHE["nc"]
    in_maps = _prep_in_maps(**inputs)
    res = run_bass_kernel_spmd(nc, in_maps, list(range(NCORES)), trace=trace)
    out = np.empty((N, DM), dtype=np.float32)
    for c in range(NCORES):
        out[128 * c:128 * (c + 1)] = res.results[c]["out"][0:128]
        out[1024 + 128 * c:1024 + 128 * (c + 1)] = res.results[c]["out"][128:256]
    return out.reshape(1, N, DM), res


def kernel(**inputs):
    out, _ = kernel_run(inputs, trace=False)
    return out



# revision 86
# speedup vs baseline: 1.2853x; 1.2853x over previous
"""HALE attention on 8 Trainium2 NeuronCores — bf16 rewrite.

Sharding: 2 heads/core (tensor-parallel), sequence-parallel tail after a
split AllToAll (core c owns rows [128c:128c+128] and [1024+128c:+128]).

Key layout choices vs the fp32 baseline:
  - all matmul operands bf16 (fp32 PSUM accumulation): single-pass MMs,
    FWL weight loads, half DMA bytes.
  - x^T staged on the host ([128, 8, 2048]); no on-device x transposes.
  - natural-orientation tensors built with PE transposes + PSUM copies
    (XBAR dma_start_transpose costs ~1.2us of issuing-queue time each and
    corrupts data for <128-partition sources, so it is not used).
  - Haar block means + level projections batched into 512-col matmuls with
    the level dim stacked along free cols.
  - augmented-value tiles padded 65->68 cols so PSUM rows stay 8B-aligned.
  - AllToAll split in two (chunks 0-7 / 8-15); first half overlaps the
    second half of the recurrence.

Scheduling notes (365us -> ~308us):
  - recurrence matmuls grouped by weight-tile shape, with the A@V / q@S
    pair interleaved across the two heads per level: consecutive
    same-shape matmuls keep LDWEIGHTS shadowed (54-107ns periods vs
    ~165ns when shapes alternate). Each PSUM region's accumulation group
    must stay contiguous per bank (opening 5 groups on one bank before
    closing any corrupts results on hardware, though the sim accepts it).
  - gpsimd cannot access PSUM and has no TensorScalarPtr; it takes the
    SBUF-only work (P-mask h1, loc-diff, a2a stores). The scalar engine
    drains pso/psSd PSUM->SBUF (bf16) so the per-level combines read
    SBUF at 2x 16-bit DVE rate.
  - startup DMAs reordered critical-path-first across the three DGE
    queues; a tiny warm-up AllToAll absorbs the CC stream's ~11.5us
    first-use setup so the real a2aA starts ~1us after trigger.
  - tail split into per-128-row passes: the gate x-part matmuls (no a2a
    dependency) and pass-0 overlap the second AllToAll; gate/out weights
    stream on the sync queue during the recurrence (16+8 resident
    tiles), with xT's 32KB freed after staging xslT to make room.
"""

import numpy as np
import ml_dtypes
from contextlib import ExitStack

import concourse.bass as bass
import concourse.bacc as bacc
import concourse.tile as tile
import concourse.mybir as mybir
from concourse.bass_utils import run_bass_kernel_spmd

F32 = mybir.dt.float32
BF16 = mybir.dt.bfloat16
AF = mybir.ActivationFunctionType
OP = mybir.AluOpType
BF = ml_dtypes.bfloat16

NCORES = 8
N = 2048
DM = 1024
H = 16
DH = 64
L = 4
CH = 128
NCH = N // CH
WIN = 64
NSL = N // NCORES
EPS = 1e-6
VA = 68  # padded aug-value width (65 used, 68 for 8B PSUM row alignment)

_CACHE = {}
DEBUG = False


def _host_constants():
    ck = np.arange(CH)[:, None]
    cq = np.arange(CH)[None, :]
    maskT = (ck <= cq).astype(np.float32)
    prev = (ck >= cq + WIN + 1).astype(np.float32)
    cur = ((ck <= cq) & (ck >= cq - (WIN - 1))).astype(np.float32)
    lmask = np.concatenate([prev, cur], axis=1)
    Ml = np.zeros((L, CH, CH), dtype=np.float32)
    for lv in range(L):
        b = 2 ** (lv + 1)
        m = np.arange(CH)[:, None]
        n = np.arange(CH)[None, :]
        Ml[lv] = np.where(((m // b) == (n // b)) & (m <= n),
                          1.0 / (n % b + 1.0), 0.0)
    maskT4 = np.tile(maskT, (1, 5))
    Ml_all = Ml.transpose(1, 0, 2).reshape(CH, L * CH)
    return maskT4, lmask, Ml_all


def _blockdiag2(a):
    z = np.zeros((128, 128), dtype=np.float32)
    z[:64, :64] = a
    z[64:, 64:] = a
    return z


def _build_nc():
    nc = bacc.Bacc("TRN2", target_bir_lowering=False, debug=False,
                   num_devices=NCORES)

    xT_d = nc.dram_tensor("xT", [128, 8 * N], BF16, kind="ExternalInput")
    wproj_d = nc.dram_tensor("wproj", [128, 3 * 8 * 128], BF16,
                             kind="ExternalInput")
    wnat_d = nc.dram_tensor("wnat", [128, 8 * 256], BF16,
                            kind="ExternalInput")
    bdWkT_d = nc.dram_tensor("bdWkT", [128, L * 128], BF16,
                             kind="ExternalInput")
    bdWvT_d = nc.dram_tensor("bdWvT", [128, L * 128], BF16,
                             kind="ExternalInput")
    Ml_d = nc.dram_tensor("Ml", [128, L * 128], BF16, kind="ExternalInput")
    maskT4_d = nc.dram_tensor("maskT4", [128, 640], BF16,
                              kind="ExternalInput")
    lmask_d = nc.dram_tensor("lmask", [128, 256], BF16, kind="ExternalInput")
    ident_d = nc.dram_tensor("ident", [128, 128], BF16, kind="ExternalInput")
    w5b_d = nc.dram_tensor("w5b", [128, 5], F32, kind="ExternalInput")
    wgT_d = nc.dram_tensor("wgT", [2 * DM, DM], BF16, kind="ExternalInput")
    woT_d = nc.dram_tensor("woT", [DM, DM], BF16, kind="ExternalInput")
    wgo8_d = nc.dram_tensor("wgo8", [128, 8], BF16, kind="ExternalInput")
    bg_d = nc.dram_tensor("bg", [1, DM], BF16, kind="ExternalInput")
    bo_d = nc.dram_tensor("bo", [1, DM], BF16, kind="ExternalInput")
    bgo_d = nc.dram_tensor("bgo", [128, 1], F32, kind="ExternalInput")
    out_d = nc.dram_tensor("out", [2 * 128, DM], F32, kind="ExternalOutput")

    dbg_d = {}
    if DEBUG:
        for nm, w in (("qT", N), ("kpT", N), ("knat", N), ("kpnat", N),
                      ("vnat", N), ("vlnat", N), ("kplT", L * N),
                      ("kplN", L * N), ("vlvA", L * NCH * 2 * VA),
                      ("vaug", NCH * 2 * VA), ("glob", N), ("loc", N),
                      ("S", 5 * VA), ("atm", 640)):
            dbg_d[nm] = nc.dram_tensor(f"dbg_{nm}", [128, w], BF16,
                                       kind="ExternalOutput")
        for nm, w in (("pso", 5 * VA), ("dmax", 5), ("rw", 5)):
            dbg_d[nm] = nc.dram_tensor(f"dbg_{nm}", [128, w], F32,
                                       kind="ExternalOutput")

    # [dest core, tensor(diff,glob), 128, 128] for chunk halves A/B
    a2aA_in = nc.dram_tensor("a2aA_in", [NCORES, 2, 128, 128], BF16)
    a2aA_out = nc.dram_tensor("a2aA_out", [NCORES, 2, 128, 128], BF16)
    a2aB_in = nc.dram_tensor("a2aB_in", [NCORES, 2, 128, 128], BF16)
    a2aB_out = nc.dram_tensor("a2aB_out", [NCORES, 2, 128, 128], BF16)
    # tiny warm-up collective to absorb the CC stream's first-use setup
    a2aW_in = nc.dram_tensor("a2aW_in", [NCORES, 16], BF16)
    a2aW_out = nc.dram_tensor("a2aW_out", [NCORES, 16], BF16)

    with tile.TileContext(nc) as tc, ExitStack() as root:
        cpool = root.enter_context(tc.tile_pool(name="consts", bufs=1))
        persist = root.enter_context(tc.tile_pool(name="persist", bufs=1))

        maskT4 = cpool.tile([128, 640], BF16)
        lmask = cpool.tile([128, 256], BF16)
        Ml_sb = cpool.tile([128, L * 128], BF16)
        bdWkT = cpool.tile([128, L * 128], BF16)
        bdWvT = cpool.tile([128, L * 128], BF16)
        ident = cpool.tile([128, 128], BF16)
        w5b = cpool.tile([128, 5], F32)
        ones_row = cpool.tile([1, 128], BF16)
        bg_sb = cpool.tile([1, DM], BF16)
        bo_sb = cpool.tile([1, DM], BF16)
        bgo_sb = cpool.tile([128, 1], F32)
        wgo_sb = cpool.tile([128, 8], BF16)

        glob = persist.tile([128, N], BF16)
        loc = persist.tile([128, N], BF16)
        S_bf = persist.tile([128, 5, VA], BF16)
        xslT = persist.tile([128, 8, 2, 128], BF16)
        diff_gs = persist.tile([128, 2, DM], BF16)
        glob_gs = persist.tile([128, 2, DM], BF16)
        recur = persist
        # xT lives in its own stack-scoped pool: freed once xslT is staged
        xtp = ExitStack()
        xT = xtp.enter_context(
            tc.tile_pool(name="xtp", bufs=1)).tile([128, 8, N], BF16)
        qT = recur.tile([128, N], BF16)
        klT = recur.tile([128, N], BF16)
        qpT = recur.tile([128, N], BF16)
        kpT = recur.tile([128, N], BF16)
        kpnat = recur.tile([128, N], BF16)
        kplT = recur.tile([128, L, N], BF16)
        kplN = recur.tile([128, L, N], BF16)
        vaug = recur.tile([128, NCH, 2, VA], BF16)
        vlaug = recur.tile([128, NCH, 2, VA], BF16)
        vlvA = recur.tile([128, L, NCH, 2, VA], BF16)
        if DEBUG:
            dpso = persist.tile([128, 5, VA], F32)
            datm = persist.tile([128, 640], BF16)
            ddmax = persist.tile([128, 5], F32)
            drw = persist.tile([128, 5], F32)

        nc.vector.memset(vaug[:, :, :, 64:65], 1.0)
        nc.vector.memset(vlaug[:, :, :, 64:65], 1.0)
        nc.vector.memset(vlvA[:, :, :, :, 64:65], 1.0)

        def cp(dst, src, on_scalar=False):
            if on_scalar:
                nc.scalar.activation(dst, src, AF.Copy)
            else:
                nc.vector.tensor_copy(dst, src)

        def cp3(dst, src, eng):
            # gpsimd cannot touch PSUM; rotate psum->sbuf copies over V/S only
            if eng % 2 == 0:
                nc.vector.tensor_copy(dst, src)
            else:
                nc.scalar.activation(dst, src, AF.Copy)

        with ExitStack() as phA:
            nat_p = phA.enter_context(tc.tile_pool(name="nat", bufs=1))
            knat = nat_p.tile([128, N], BF16)
            vnat = nat_p.tile([128, N], BF16)
            vlnat = nat_p.tile([128, N], BF16)
            tmp_p = phA.enter_context(tc.tile_pool(name="phitmp", bufs=2))
            phPrj = phA.enter_context(ExitStack())
            prj = phPrj.enter_context(tc.tile_pool(name="prj", bufs=1))
            phP = phA.enter_context(ExitStack())
            ps_pr = phP.enter_context(
                tc.tile_pool(name="ps_pr", bufs=2, space="PSUM"))

            wproj = prj.tile([128, 3 * 8 * 128], BF16)
            wnat = prj.tile([128, 8, 256], BF16)
            kT = prj.tile([128, N], BF16)

            # ---- DMA issue order: critical-path loads first, spread over
            # the DGE queues; per-projection wproj slices so the first
            # matmul only waits on 0.4MB, not the whole tensor.
            for ip in range(3):
                nc.sync.dma_start(wproj[:, 1024 * ip:1024 * (ip + 1)],
                                  wproj_d[:, 1024 * ip:1024 * (ip + 1)])
            nc.sync.dma_start(wnat[:], wnat_d[:])
            xq = (nc.scalar, nc.gpsimd, nc.sync)
            for k in range(8):
                xq[k % 3].dma_start(xT[:, k, :], xT_d[:, N * k:N * (k + 1)])
            nc.sync.dma_start(ident[:], ident_d[:])
            nc.sync.dma_start(Ml_sb[:], Ml_d[:])
            nc.scalar.dma_start(maskT4[:], maskT4_d[:])
            nc.scalar.dma_start(lmask[:], lmask_d[:])
            nc.gpsimd.dma_start(bdWkT[:], bdWkT_d[:])
            nc.gpsimd.dma_start(bdWvT[:], bdWvT_d[:])
            nc.gpsimd.dma_start(w5b[:], w5b_d[:])
            nc.scalar.dma_start(bg_sb[:], bg_d[:])
            nc.scalar.dma_start(bo_sb[:], bo_d[:])
            nc.gpsimd.dma_start(bgo_sb[:], bgo_d[:])
            nc.gpsimd.dma_start(wgo_sb[:], wgo8_d[:])
            nc.vector.memset(ones_row[:], 1.0)
            nc.gpsimd.collective_compute(
                "AllToAll", OP.bypass,
                ins=[a2aW_in.ap().opt()], outs=[a2aW_out.ap().opt()],
                replica_groups=[list(range(NCORES))])

            def phi_big(dst, src):
                tmp = tmp_p.tile([128, N], BF16, tag="phitmp")
                nc.vector.tensor_scalar_min(tmp[:], src[:], 0.0)
                nc.scalar.activation(dst[:], tmp[:], AF.Exp)
                nc.vector.scalar_tensor_tensor(
                    dst[:], src[:], 0.0, dst[:], op0=OP.max, op1=OP.add)

            # ----- projections (k-outer, 4x512-col accumulators) -----
            for ip, dstT in enumerate((qT, kT, klT)):
                accs = [ps_pr.tile([128, 512], F32, tag=f"pa{nb}",
                                   name=f"acc{nb}")
                        for nb in range(4)]
                for k in range(8):
                    for nb in range(4):
                        nc.tensor.matmul(
                            accs[nb][:], wproj[:, (ip * 8 + k) * 128:
                                                (ip * 8 + k + 1) * 128],
                            xT[:, k, 512 * nb:512 * (nb + 1)],
                            start=(k == 0), stop=(k == 7))
                for nb in range(4):
                    cp(dstT[:, 512 * nb:512 * (nb + 1)], accs[nb][:],
                       on_scalar=(nb % 2 == 1))
                if ip == 0:
                    phi_big(qpT, qT)
                elif ip == 1:
                    phi_big(kpT, kT)


            # ----- Haar: block means + level projections (batched) -----
            phP.close()
            with ExitStack() as trA:
                # v / v_loc computed directly in natural [seq, d] orientation
                # (x^T chunk as stationary weights, W columns moving): no
                # transposes, no extra PSUM round-trips.
                ps_nat = trA.enter_context(
                    tc.tile_pool(name="ps_nat", bufs=3, space="PSUM"))
                ps_tr1 = trA.enter_context(
                    tc.tile_pool(name="ps_tr1", bufs=3, space="PSUM"))
                for i in range(NCH):
                    sl = slice(CH * i, CH * (i + 1))
                    pn = ps_nat.tile([128, 256], F32, tag="nat")
                    for k in range(8):
                        nc.tensor.matmul(pn[:], xT[:, k, sl], wnat[:, k, :],
                                         start=(k == 0), stop=(k == 7))
                    cp3(vnat[:, sl], pn[:, 0:128], i)
                    cp3(vlnat[:, sl], pn[:, 128:256], i + 1)
                    ptn = ps_tr1.tile([128, 128], BF16, tag="ptk")
                    nc.tensor.transpose(ptn[:], kT[:, sl], ident[:])
                    cp3(knat[:, sl], ptn[:], i)
                    nc.gpsimd.tensor_copy(
                        vaug[:, i, :, 0:64],
                        vnat[:, sl].rearrange("p (h d) -> p h d", h=2))
                    nc.gpsimd.tensor_copy(
                        vlaug[:, i, :, 0:64],
                        vlnat[:, sl].rearrange("p (h d) -> p h d", h=2))
            bm_p = phA.enter_context(tc.tile_pool(name="bm", bufs=2))
            ps_bm = phA.enter_context(
                tc.tile_pool(name="ps_bm", bufs=1, space="PSUM"))
            ps_trH = phA.enter_context(
                tc.tile_pool(name="ps_trH", bufs=2, space="PSUM"))
            for g in range(4):
                bmk = bm_p.tile([128, 4, L, 128], BF16, tag="bmk", bufs=2)
                bmv = bm_p.tile([128, 4, L, 128], BF16, tag="bmv", bufs=2)
                for ig in range(4):
                    i = 4 * g + ig
                    sl = slice(CH * i, CH * (i + 1))
                    psk = ps_bm.tile([128, 512], F32, tag="bmk")
                    nc.tensor.matmul(psk[:], knat[:, sl], Ml_sb[:],
                                     start=True, stop=True)
                    nc.vector.tensor_copy(bmk[:, ig], psk[:].rearrange(
                        "p (l s) -> p l s", l=L))
                    psv = ps_bm.tile([128, 512], F32, tag="bmv")
                    nc.tensor.matmul(psv[:], vnat[:, sl], Ml_sb[:],
                                     start=True, stop=True)
                    nc.scalar.activation(bmv[:, ig], psv[:].rearrange(
                        "p (l s) -> p l s", l=L), AF.Copy)
                nsl = slice(512 * g, 512 * (g + 1))
                for lv in range(L):
                    pst = ps_bm.tile([128, 512], F32, tag="kt", bufs=1)
                    nc.tensor.matmul(pst[:],
                                     bdWkT[:, 128 * lv:128 * (lv + 1)],
                                     bmk[:, :, lv, :], start=True, stop=True)
                    tmp = tmp_p.tile([128, 512], BF16, tag="phs")
                    nc.vector.tensor_scalar_min(tmp[:], pst[:], 0.0)
                    nc.scalar.activation(kplT[:, lv, nsl], tmp[:], AF.Exp)
                    nc.vector.scalar_tensor_tensor(
                        kplT[:, lv, nsl], pst[:], 0.0, kplT[:, lv, nsl],
                        op0=OP.max, op1=OP.add)
                for lv in range(L):
                    for ig in range(4):
                        i = 4 * g + ig
                        sl = slice(CH * i, CH * (i + 1))
                        ptk = ps_trH.tile([128, 128], BF16, tag="ptk")
                        nc.tensor.transpose(ptk[:], kplT[:, lv, sl], ident[:])
                        cp3(kplN[:, lv, sl], ptk[:], 2 * ig)
                        # vlv directly in natural [seq, d] orientation:
                        # contraction over the block-mean feature partitions
                        pv = ps_trH.tile([128, 128], F32, tag="ptv")
                        nc.tensor.matmul(pv[:], bmv[:, ig, lv, :],
                                         bdWvT[:, 128 * lv:128 * (lv + 1)],
                                         start=True, stop=True)
                        cp3(vlvA[:, lv, i, :, 0:64],
                            pv[:].rearrange("p (h d) -> p h d", h=2),
                            2 * ig + 1)
            # kpnat is only consumed by the phase-B state updates: doing the
            # phi here keeps it off the Haar section's vector critical path
            phi_big(kpnat, knat)

        # stage this core's x^T rows, then free xT's SBUF for the tail
        pid = nc.sync.partition_id()
        r0 = pid * 128
        xTv = xT[:].rearrange("p k (t n) -> p k t n", t=2)
        nc.sync.dma_start(xslT[:], xTv[:, :, :, bass.ds(r0, 128)])
        xtp.close()

        # ----- chunk-major recurrence + local attention -----
        with ExitStack() as phB:
            atm_p = phB.enter_context(tc.tile_pool(name="atm", bufs=3))
            tin_p = phB.enter_context(tc.tile_pool(name="tiny", bufs=4))
            ps_Aa = phB.enter_context(
                tc.tile_pool(name="ps_Aa", bufs=2, space="PSUM"))
            ps_Ab = phB.enter_context(
                tc.tile_pool(name="ps_Ab", bufs=1, space="PSUM"))
            ps_O = phB.enter_context(
                tc.tile_pool(name="ps_O", bufs=2, space="PSUM"))
            ps_Sd = phB.enter_context(
                tc.tile_pool(name="ps_Sd", bufs=1, space="PSUM"))

            def emit_scores(i):
                # scores + masks for chunk i; emitted one chunk ahead so the
                # next chunk's score matmuls hide this chunk's mask latency
                sl = slice(CH * i, CH * (i + 1))
                atms, Ps = [], []
                for h in range(2):
                    hp = slice(64 * h, 64 * h + 64)
                    psa_a = ps_Aa.tile([128, 512], F32, tag="psa", bufs=3)
                    psab = ps_Ab.tile([128, 384], F32, tag="psb", bufs=2)
                    for lv in range(5):
                        lhsT = (kpT[hp, sl] if lv == 0
                                else kplT[hp, lv - 1, sl])
                        out = (psa_a[:, 128 * (lv - 1):128 * lv] if lv > 0
                               else psab[:, 0:128])
                        nc.tensor.matmul(out, lhsT, qpT[hp, sl],
                                         start=True, stop=True)
                    if i > 0:
                        nc.tensor.matmul(psab[:, 128:256],
                                         klT[hp, CH * (i - 1):CH * i],
                                         qT[hp, sl], start=True, stop=True)
                    nc.tensor.matmul(psab[:, 256:384], klT[hp, sl],
                                     qT[hp, sl], start=True, stop=True)
                    ve = nc.vector if h == 0 else nc.gpsimd
                    atm = atm_p.tile([128, 640], BF16, tag="atm")
                    nc.vector.tensor_mul(atm[:, 0:512], psa_a[:],
                                         maskT4[:, 0:512])
                    nc.vector.tensor_mul(atm[:, 512:640], psab[:, 0:128],
                                         maskT4[:, 512:640])
                    P = atm_p.tile([128, 256], BF16, tag="P")
                    if i > 0:
                        nc.scalar.activation(P[:], psab[:, 128:384], AF.Exp,
                                             scale=0.125)
                        ve.tensor_mul(P[:], P[:], lmask[:])
                    else:
                        nc.scalar.activation(P[:, 128:256], psab[:, 256:384],
                                             AF.Exp, scale=0.125)
                        ve.tensor_mul(P[:, 128:256], P[:, 128:256],
                                      lmask[:, 128:256])
                    atms.append(atm)
                    Ps.append(P)
                return atms, Ps

            pending = emit_scores(0)
            for i in range(NCH):
                sl = slice(CH * i, CH * (i + 1))
                psSd = ps_Sd.tile([128, 7, VA], F32, tag="psSd")
                atms, Ps = pending
                if i + 1 < NCH:
                    pending = emit_scores(i + 1)
                # ---- A@V + q@S: cross-head interleave per level. Each PSUM
                # region's accumulation group stays contiguous on its bank
                # (hw requirement), but pairing the two heads halves the
                # weight-shape alternation so LDWEIGHTS shadows better.
                psos = [ps_O.tile([128, 5, VA], F32, tag="pso",
                                  name=f"pso{h}", bufs=2) for h in range(2)]
                for lv in range(5):
                    for h in range(2):
                        atm = atms[h]
                        alv = atm[:, 512:640] if lv == 0 else \
                            atm[:, 128 * (lv - 1):128 * lv]
                        va_l = (vaug[:, i, h, 0:65] if lv == 0
                                else vlvA[:, lv - 1, i, h, 0:65])
                        nc.tensor.matmul(psos[h][:, lv, 0:65], alv, va_l,
                                         start=True, stop=(i == 0))
                    if i > 0:
                        for h in range(2):
                            hp = slice(64 * h, 64 * h + 64)
                            nc.tensor.matmul(psos[h][:, lv, 0:65],
                                             qpT[hp, sl], S_bf[hp, lv, 0:65],
                                             start=False, stop=True)
                # group 3: state updates, 128x64 weights (10 instr)
                for h in range(2):
                    hp = slice(64 * h, 64 * h + 64)
                    c0 = CH * i + 64 * h
                    for lv in range(5):
                        kn_l = (kpnat[:, c0:c0 + 64] if lv == 0
                                else kplN[:, lv - 1, c0:c0 + 64])
                        va_l = (vaug[:, i, h, 0:65] if lv == 0
                                else vlvA[:, lv - 1, i, h, 0:65])
                        nc.tensor.matmul(psSd[hp, lv, 0:65], kn_l, va_l,
                                         start=True, stop=True)
                # group 4: local attention out, 128x128 weights (2-4 instr)
                for h in range(2):
                    P = Ps[h]
                    if i > 0:
                        nc.tensor.matmul(psSd[:, 5 + h, 0:65], P[:, 0:128],
                                         vlaug[:, i - 1, h, 0:65],
                                         start=True, stop=False)
                    nc.tensor.matmul(psSd[:, 5 + h, 0:65], P[:, 128:256],
                                     vlaug[:, i, h, 0:65],
                                     start=(i == 0), stop=True)
                # ---- per-head normalization / combine (vector+gpsimd) -----
                for h in range(2):
                    ve = nc.vector if h == 0 else nc.gpsimd
                    c0 = CH * i + 64 * h
                    pso = psos[h]
                    # drain PSUM->SBUF on the scalar engine, combine on V/G
                    po_t = tin_p.tile([128, 5, VA], BF16, tag=f"pot{h}")
                    nc.scalar.activation(po_t[:, :, 0:65], pso[:, :, 0:65],
                                         AF.Copy)
                    lo_t = tin_p.tile([128, VA], BF16, tag=f"lot{h}")
                    nc.scalar.activation(lo_t[:, 0:65], psSd[:, 5 + h, 0:65],
                                         AF.Copy)
                    dmax = tin_p.tile([128, 5], F32, tag=f"dmax{h}")
                    if DEBUG and i == 1 and h == 0:
                        nc.vector.tensor_copy(dpso[:], pso[:])
                        nc.vector.tensor_copy(datm[:], atms[h][:])
                    nc.vector.tensor_scalar_max(dmax[:], po_t[:, :, 64], EPS)
                    rec = tin_p.tile([128, 5], F32, tag=f"rec{h}")
                    nc.vector.reciprocal(rec[:], dmax[:])
                    rw = tin_p.tile([128, 5], F32, tag=f"rw{h}")
                    ve.tensor_mul(rw[:], rec[:], w5b[:])
                    if DEBUG and i == 1 and h == 0:
                        nc.vector.tensor_copy(ddmax[:], dmax[:])
                        nc.vector.tensor_copy(drw[:], rw[:])
                    gsl = glob[:, c0:c0 + 64]
                    nc.vector.tensor_scalar_mul(gsl, po_t[:, 0, 0:64],
                                                rw[:, 0:1])
                    for lv in range(1, 5):
                        nc.vector.scalar_tensor_tensor(
                            gsl, po_t[:, lv, 0:64], rw[:, lv:lv + 1], gsl,
                            op0=OP.mult, op1=OP.add)
                    dm = tin_p.tile([128, 1], F32, tag=f"dm{h}")
                    nc.vector.tensor_scalar_max(dm[:], lo_t[:, 64:65], 1e-30)
                    rl = tin_p.tile([128, 1], F32, tag=f"rl{h}")
                    nc.vector.reciprocal(rl[:], dm[:])
                    nc.vector.tensor_scalar_mul(loc[:, c0:c0 + 64],
                                                lo_t[:, 0:64], rl[:])
                if i == 0:
                    nc.vector.tensor_copy(S_bf[:, :, 0:65],
                                          psSd[:, 0:5, 0:65])
                else:
                    nc.vector.tensor_add(S_bf[:, :, 0:65], S_bf[:, :, 0:65],
                                         psSd[:, 0:5, 0:65])
                # diff in place + stage this chunk for its dest core
                nc.gpsimd.tensor_sub(loc[:, sl], loc[:, sl], glob[:, sl])
                dst = a2aA_in if i < 8 else a2aB_in
                nc.gpsimd.dma_start(dst.ap()[i % 8, 0], loc[:, sl])
                nc.gpsimd.dma_start(dst.ap()[i % 8, 1], glob[:, sl])
                if i == 7:
                    nc.gpsimd.collective_compute(
                        "AllToAll", OP.bypass,
                        ins=[a2aA_in.ap().opt()], outs=[a2aA_out.ap().opt()],
                        replica_groups=[list(range(NCORES))])
            nc.gpsimd.collective_compute(
                "AllToAll", OP.bypass,
                ins=[a2aB_in.ap().opt()], outs=[a2aB_out.ap().opt()],
                replica_groups=[list(range(NCORES))])
            nc.gpsimd.dma_start(
                diff_gs[:, 1, :].rearrange("p (s m) -> p s m", s=8),
                a2aB_out.ap()[:, 0].rearrange("s p m -> p s m"))
            nc.gpsimd.dma_start(
                glob_gs[:, 1, :].rearrange("p (s m) -> p s m", s=8),
                a2aB_out.ap()[:, 1].rearrange("s p m -> p s m"))

        # ---------- sequence-parallel tail (split per 128-row half) -------
        with ExitStack() as phC:
            tl = phC.enter_context(tc.tile_pool(name="tail", bufs=1))
            wst = phC.enter_context(tc.tile_pool(name="wstream", bufs=8))
            ps_tr2 = phC.enter_context(
                tc.tile_pool(name="ps_tr2", bufs=2, space="PSUM"))
            ps_g = phC.enter_context(
                tc.tile_pool(name="ps_g", bufs=1, space="PSUM"))

            # weight streams: wg ring (8 deep, 16 slices) + wo (8 alive).
            # The DMAs sit on the sync queue, which runs ahead during the
            # recurrence, so the first ring fill overlaps phase B entirely.
            wg_ts = {}
            wo_ts = {}

            # x-part of the gate GEMM for both halves: no a2a dependency,
            # fills the PE while the second AllToAll is in flight.
            psGs = []
            for t2 in range(2):
                psG = [ps_g.tile([128, 512], F32, tag=f"psG{t2}{j}",
                                 name=f"psG{t2}{j}") for j in range(2)]
                psGs.append(psG)
                for kc in range(8):
                    if t2 == 0:
                        wg_t = wst.tile([128, DM], BF16, tag="wg", bufs=16,
                                        name=f"wg{kc}")
                        nc.sync.dma_start(
                            wg_t[:], wgT_d[128 * kc:128 * (kc + 1), :])
                        wg_ts[kc] = wg_t
                    lhs = xslT[:, kc, t2, :]
                    for g2 in range(2):
                        nc.tensor.matmul(
                            psG[g2][:], lhs,
                            wg_ts[kc][:, 512 * g2:512 * (g2 + 1)],
                            start=(kc == 0), stop=False)

            # gather this core's first-half rows as soon as the first
            # collective lands (sync queue, behind the first wg batch)
            nc.sync.dma_start(
                diff_gs[:, 0, :].rearrange("p (s m) -> p s m", s=8),
                a2aA_out.ap()[:, 0].rearrange("s p m -> p s m"))
            nc.sync.dma_start(
                glob_gs[:, 0, :].rearrange("p (s m) -> p s m", s=8),
                a2aA_out.ap()[:, 1].rearrange("s p m -> p s m"))

            for t2 in range(2):
                psG = psGs[t2]

                diffT = tl.tile([128, 8, 128], BF16, tag="dT", name=f"diffT{t2}")
                for k in range(8):
                    pt = ps_tr2.tile([128, 128], BF16, tag="ptr2")
                    nc.tensor.transpose(
                        pt[:], diff_gs[:, t2, 128 * k:128 * (k + 1)],
                        ident[:])
                    cp(diffT[:, k, :], pt[:], on_scalar=(k % 2 == 1))
                for kc in range(8, 16):
                    if t2 == 0:
                        wg_t = wst.tile([128, DM], BF16, tag="wg", bufs=16,
                                        name=f"wg{kc}")
                        nc.sync.dma_start(
                            wg_t[:], wgT_d[128 * kc:128 * (kc + 1), :])
                        wg_ts[kc] = wg_t
                    lhs = diffT[:, kc - 8, :]
                    for g2 in range(2):
                        nc.tensor.matmul(
                            psG[g2][:], lhs,
                            wg_ts[kc][:, 512 * g2:512 * (g2 + 1)],
                            start=False, stop=False)
                gh = tl.tile([128, DM], BF16, tag="gh", name=f"gh{t2}")
                for g2 in range(2):
                    nc.tensor.matmul(
                        psG[g2][:], ones_row[:],
                        bg_sb[:, 512 * g2:512 * (g2 + 1)],
                        start=False, stop=True)
                    nc.scalar.activation(
                        gh[:, 512 * g2:512 * (g2 + 1)], psG[g2][:], AF.Silu)
                ghT = tl.tile([128, 8, 128], BF16, tag="ghT", name=f"ghT{t2}")
                for k in range(8):
                    pt = ps_tr2.tile([128, 128], BF16, tag="ptr2")
                    nc.tensor.transpose(
                        pt[:], gh[:, 128 * k:128 * (k + 1)], ident[:])
                    cp(ghT[:, k, :], pt[:], on_scalar=(k % 2 == 1))
                psAl = ps_tr2.tile([128, 1], F32, tag="psAl")
                for gc in range(8):
                    nc.tensor.matmul(psAl[:, 0:1], ghT[:, gc, :],
                                     wgo_sb[:, gc:gc + 1],
                                     start=(gc == 0), stop=(gc == 7))
                alpha = tl.tile([128, 1], F32, tag="al", name=f"alpha{t2}")
                nc.scalar.activation(alpha[:], psAl[:], AF.Sigmoid,
                                     bias=bgo_sb[:])
                mx = diff_gs[:, t2, :]
                nc.vector.scalar_tensor_tensor(
                    mx, diff_gs[:, t2, :], alpha[:, 0:1], glob_gs[:, t2, :],
                    op0=OP.mult, op1=OP.add)
                mxT = tl.tile([128, 8, 128], BF16, tag="mxT", name=f"mxT{t2}")
                for k in range(8):
                    pt = ps_tr2.tile([128, 128], BF16, tag="ptr2")
                    nc.tensor.transpose(
                        pt[:], diff_gs[:, t2, 128 * k:128 * (k + 1)],
                        ident[:])
                    cp(mxT[:, k, :], pt[:], on_scalar=(k % 2 == 1))
                out_sb = tl.tile([128, DM], F32, tag="out", name=f"out{t2}")
                psF = [ps_g.tile([128, 512], F32, tag=f"psG{t2}{j}",
                                 name=f"psF{t2}{j}") for j in range(2)]
                for kc in range(8):
                    if t2 == 0:
                        wo_t = wst.tile([128, DM], BF16, tag="wo", bufs=8,
                                        name=f"wo{kc}")
                        nc.sync.dma_start(
                            wo_t[:], woT_d[128 * kc:128 * (kc + 1), :])
                        wo_ts[kc] = wo_t
                    for o2 in range(2):
                        nc.tensor.matmul(
                            psF[o2][:], mxT[:, kc, :],
                            wo_ts[kc][:, 512 * o2:512 * (o2 + 1)],
                            start=(kc == 0), stop=False)
                for o2 in range(2):
                    nc.tensor.matmul(
                        psF[o2][:], ones_row[:],
                        bo_sb[:, 512 * o2:512 * (o2 + 1)],
                        start=False, stop=True)
                    cp(out_sb[:, 512 * o2:512 * (o2 + 1)], psF[o2][:],
                       on_scalar=(o2 == 1))
                nc.sync.dma_start(out_d.ap()[128 * t2:128 * (t2 + 1), :],
                                  out_sb[:])

        if DEBUG:
            for nm, t in (("qT", qT), ("kpT", kpT), ("knat", knat),
                          ("kpnat", kpnat), ("vnat", vnat), ("vlnat", vlnat),
                          ("glob", glob), ("loc", loc)):
                nc.gpsimd.dma_start(dbg_d[nm].ap(), t[:])
            nc.gpsimd.dma_start(
                dbg_d["kplT"].ap().rearrange("p (l n) -> p l n", l=L), kplT[:])
            nc.gpsimd.dma_start(
                dbg_d["kplN"].ap().rearrange("p (l n) -> p l n", l=L), kplN[:])
            nc.gpsimd.dma_start(
                dbg_d["vlvA"].ap().rearrange(
                    "p (l c h v) -> p l c h v", l=L, c=NCH, h=2), vlvA[:])
            nc.gpsimd.dma_start(
                dbg_d["vaug"].ap().rearrange(
                    "p (c h v) -> p c h v", c=NCH, h=2), vaug[:])
            nc.gpsimd.dma_start(
                dbg_d["S"].ap().rearrange("p (l v) -> p l v", l=5), S_bf[:])
            nc.gpsimd.dma_start(
                dbg_d["pso"].ap().rearrange("p (l v) -> p l v", l=5), dpso[:])
            nc.gpsimd.dma_start(dbg_d["atm"].ap(), datm[:])
            nc.gpsimd.dma_start(dbg_d["dmax"].ap(), ddmax[:])
            nc.gpsimd.dma_start(dbg_d["rw"].ap(), drw[:])

    nc.compile()
    return nc


def _prep_in_maps(x, Wq, Wk, Wv, Wkl, Wvl, haar_Wk, haar_Wv, haar_scale,
                  Wg, bg, Wgo, bgo, Wo, bo):
    maskT4, lmask, Ml_all = _host_constants()
    x2 = np.asarray(x, dtype=np.float32).reshape(N, DM)
    xT = np.ascontiguousarray(
        x2.reshape(N, 8, 128).transpose(2, 1, 0).reshape(128, 8 * N)
    ).astype(BF)
    bdWkT = np.concatenate(
        [_blockdiag2(np.asarray(haar_Wk[lv], dtype=np.float32).T)
         for lv in range(L)], axis=1)
    bdWvT = np.concatenate(
        [_blockdiag2(np.asarray(haar_Wv[lv], dtype=np.float32).T)
         for lv in range(L)], axis=1)
    hs = np.asarray(haar_scale, dtype=np.float64)
    sw = np.exp(hs - hs.max())
    sw = (sw / sw.sum()).astype(np.float32)
    w5b = np.tile(np.concatenate([[1.0], sw]).astype(np.float32)[None, :],
                  (128, 1))
    shared = {
        "xT": xT,
        "bdWkT": bdWkT.astype(BF), "bdWvT": bdWvT.astype(BF),
        "Ml": Ml_all.astype(BF), "maskT4": maskT4.astype(BF),
        "lmask": lmask.astype(BF),
        "ident": np.eye(128, dtype=np.float32).astype(BF),
        "w5b": w5b,
        "wgT": np.ascontiguousarray(
            np.asarray(Wg, dtype=np.float32).T).astype(BF),
        "woT": np.ascontiguousarray(
            np.asarray(Wo, dtype=np.float32).T).astype(BF),
        "wgo8": np.ascontiguousarray(
            np.asarray(Wgo, dtype=np.float32).reshape(8, 128).T).astype(BF),
        "bg": np.asarray(bg, dtype=np.float32).reshape(1, DM).astype(BF),
        "bo": np.asarray(bo, dtype=np.float32).reshape(1, DM).astype(BF),
        "bgo": np.full((128, 1), np.asarray(bgo, dtype=np.float32).reshape(()),
                       dtype=np.float32),
    }
    in_maps = []
    for c in range(NCORES):
        sc = slice(128 * c, 128 * (c + 1))
        m = dict(shared)
        wp = np.empty((128, 3, 8, 128), dtype=np.float32)
        for ip, W in enumerate((Wq, Wk, Wkl)):
            Wc = np.asarray(W, dtype=np.float32)[sc, :]  # [128 m, 1024 dm]
            wp[:, ip] = Wc.reshape(128, 8, 128).transpose(2, 1, 0)
        m["wproj"] = np.ascontiguousarray(
            wp.reshape(128, 3 * 8 * 128)).astype(BF)
        wn = np.empty((128, 8, 256), dtype=np.float32)
        for j, W in enumerate((Wv, Wvl)):
            Wc = np.asarray(W, dtype=np.float32)[sc, :]  # [128 e, 1024 dm]
            wn[:, :, 128 * j:128 * (j + 1)] = \
                Wc.reshape(128, 8, 128).transpose(2, 1, 0)
        m["wnat"] = np.ascontiguousarray(
            wn.reshape(128, 8 * 256)).astype(BF)
        in_maps.append(m)
    return in_maps


def kernel_run(inputs, trace=False):
    if "nc" not in _CACHE:
        _CACHE["nc"] = _build_nc()
    nc = _CACHE["nc"]
    in_maps = _prep_in_maps(**inputs)
    res = run_bass_kernel_spmd(nc, in_maps, list(range(NCORES)), trace=trace)
    out = np.empty((N, DM), dtype=np.float32)
    for c in range(NCORES):
        out[128 * c:128 * (c + 1)] = res.results[c]["out"][0:128]
        out[1024 + 128 * c:1024 + 128 * (c + 1)] = res.results[c]["out"][128:256]
    return out.reshape(1, N, DM), res


def kernel(**inputs):
    out, _ = kernel_run(inputs, trace=False)
    return out

